# revision 1
# baseline (speedup 1.0000x reference)
"""GQA attention kernel for Trainium2 (8 NeuronCores).

Sharding: batch x head-group tensor parallel. Core c handles batch (c % 2)
and head group (c // 2): 8 q heads + 2 kv heads of that batch. Each core
computes its partial o-proj output (contraction over its 512 attn features);
the host sums the 4 partials per batch.

Device-side layouts (per core):
  xT   [H=2048 hidden, S=2048 tokens] bf16  (x transposed on host)
  Q^T  [dim, tokens] per head-pair tile [128, S]   (projection with W stationary)
  K^T  same, then zero-padded to 4 variants for K=128 score matmuls
  V    [tokens, dim] natural layout (projection with xT stationary), with an
       appended ones-column -> PV matmul also yields the softmax denominator.
  scores computed transposed: S^T[kv, q] = K^T.T @ Q^T, exp'd on ACT engine,
       multiplied by a causal 0/1 mask on the diagonal tiles only,
       then attn^T[d, q] accumulated over kv tiles with V stationary.
  RoPE: rot_half(q) is a fixed 128x128 rotation matmul on PE, combined with
       cos/sin tables on DVE. The 1/sqrt(64) score scale is folded into Wq
       on the host.
"""

import os
import numpy as np
import ml_dtypes
from contextlib import ExitStack

import concourse.bass as bass
import concourse.tile as tile
from concourse import bacc
from concourse import mybir
from concourse import bass_utils

BF16 = mybir.dt.bfloat16
F32 = mybir.dt.float32
BF = ml_dtypes.bfloat16
AF = mybir.ActivationFunctionType
OP = mybir.AluOpType

H = 2048
S = 2048
B = 2
D = 64
QH = 8            # q heads per core
KVH = 2           # kv heads per core
QF = QH * D       # 512 q features per core
KF = KVH * D      # 128 kv features per core
NK = H // 128     # 16 contraction tiles
NT = S // 128     # 16 token tiles
QBS = 512         # q block size
NQB = S // QBS    # 4 q blocks
NPAIR = QF // 128 # 4 q head-pair tiles

_CACHE = {}


def _build_program():
    nc = bacc.Bacc(
        "TRN2",
        target_bir_lowering=False,
        debug=False,
        enable_asserts=False,
        num_devices=8,
    )
    xT = nc.dram_tensor("xT", [H, S], BF16, kind="ExternalInput").ap()
    wqT = nc.dram_tensor("wqT", [H, QF], BF16, kind="ExternalInput").ap()
    wkT = nc.dram_tensor("wkT", [H, KF], BF16, kind="ExternalInput").ap()
    wvT = nc.dram_tensor("wvT", [H, KF], BF16, kind="ExternalInput").ap()
    woT = nc.dram_tensor("woT", [QF, H], BF16, kind="ExternalInput").ap()
    cost = nc.dram_tensor("cost", [128, S], BF16, kind="ExternalInput").ap()
    sint = nc.dram_tensor("sint", [128, S], BF16, kind="ExternalInput").ap()
    rotT = nc.dram_tensor("rotT", [128, 128], BF16, kind="ExternalInput").ap()
    maskd = nc.dram_tensor("maskd", [128, 4 * QBS], BF16, kind="ExternalInput").ap()
    out = nc.dram_tensor("out", [S, H], F32, kind="ExternalOutput").ap()
    dbg = {}
    if os.environ.get("KERNEL_DEBUG"):
        for nm in ("d_qt0", "d_ktp00", "d_ktp10", "d_att0", "d_att1"):
            dbg[nm] = nc.dram_tensor(nm, [128, S], BF16, kind="ExternalOutput").ap()
        dbg["d_va0"] = nc.dram_tensor("d_va0", [128, NT * (D + 1)], BF16, kind="ExternalOutput").ap()

    with tile.TileContext(nc) as tc:
        with ExitStack() as ctx:
            E = ctx.enter_context
            persist = E(tc.tile_pool(name="persist", bufs=1))
            ps512 = E(tc.tile_pool(name="ps512", bufs=3, space="PSUM"))
            psAT = E(tc.tile_pool(name="psAT", bufs=3, space="PSUM"))
            psO = E(tc.tile_pool(name="psO", bufs=2, space="PSUM"))
            wk = E(tc.tile_pool(name="wk", bufs=3))
            wk2 = E(tc.tile_pool(name="wk2", bufs=2))

            # ---------------- constant loads ----------------
            wq_sb = []
            wk_sb = []
            wv_sb = []
            for k in range(NK):
                tq = persist.tile([128, QF], BF16, tag=f"wq{k}", name=f"wq{k}")
                nc.sync.dma_start(tq[:], wqT[k * 128:(k + 1) * 128, :])
                wq_sb.append(tq)
                tk = persist.tile([128, KF], BF16, tag=f"wk{k}", name=f"wk{k}")
                nc.sync.dma_start(tk[:], wkT[k * 128:(k + 1) * 128, :])
                wk_sb.append(tk)
                tv = persist.tile([128, KF], BF16, tag=f"wv{k}", name=f"wv{k}")
                nc.sync.dma_start(tv[:], wvT[k * 128:(k + 1) * 128, :])
                wv_sb.append(tv)
            rt = persist.tile([128, 128], BF16, tag="rt")
            nc.sync.dma_start(rt[:], rotT[:, :])
            cs = persist.tile([128, S], BF16, tag="cs")
            nc.sync.dma_start(cs[:], cost[:, :])
            sn = persist.tile([128, S], BF16, tag="sn")
            nc.sync.dma_start(sn[:], sint[:, :])
            msk = persist.tile([128, 4 * QBS], BF16, tag="msk")
            nc.sync.dma_start(msk[:], maskd[:, :])
            xT_sb = []
            for k in range(NK):
                t = persist.tile([128, S], BF16, tag=f"xT{k}", name=f"xT{k}")
                nc.sync.dma_start(t[:], xT[k * 128:(k + 1) * 128, :])
                xT_sb.append(t)
            wo_sb = []
            for p in range(NPAIR):
                t = persist.tile([128, H], BF16, tag=f"wo{p}", name=f"wo{p}")
                nc.sync.dma_start(t[:], woT[p * 128:(p + 1) * 128, :])
                wo_sb.append(t)

            # ---------------- persistent activation tiles ----------------
            qt_sb = [persist.tile([128, S], BF16, tag=f"qt{p}", name=f"qt{p}") for p in range(NPAIR)]
            # padded K^T variants: key (side, kvhead): side 0 -> rows 0:64 hold
            # K^T, rows 64:128 zero; side 1 -> rows 64:128 hold K^T.
            ktp = {(sd, v): persist.tile([128, S], BF16, tag=f"ktp{sd}{v}", name=f"ktp{sd}{v}")
                   for sd in (0, 1) for v in (0, 1)}
            va = [persist.tile([128, NT, D + 1], BF16, tag=f"va{v}", name=f"va{v}") for v in (0, 1)]
            att = [persist.tile([128, S], BF16, tag=f"att{p}", name=f"att{p}") for p in range(NPAIR)]

            tbc = lambda tb: slice(tb * QBS, (tb + 1) * QBS)

            def rope(ps, tb, outs):
                """ps: psum [128,512] raw pre-RoPE projection (feature-major).
                outs: list of (row_slice, out_ap) receiving rotated bf16."""
                raw = wk.tile([128, QBS], BF16, tag="rope_raw")
                nc.scalar.activation(raw[:], ps[:], AF.Copy)
                rp = ps512.tile([128, QBS], F32, tag="ps")
                nc.tensor.matmul(rp[:], lhsT=rt[:], rhs=raw[:], start=True, stop=True)
                t1 = wk.tile([128, QBS], BF16, tag="rope_t1")
                nc.vector.tensor_tensor(out=t1[:], in0=rp[:], in1=sn[:, tbc(tb)], op=OP.mult)
                t2 = wk.tile([128, QBS], BF16, tag="rope_t2")
                nc.vector.tensor_tensor(out=t2[:], in0=raw[:], in1=cs[:, tbc(tb)], op=OP.mult)
                for rows, out_ap in outs:
                    nc.vector.tensor_tensor(
                        out=out_ap, in0=t1[rows, :], in1=t2[rows, :], op=OP.add)

            # ---------------- Q projection + RoPE ----------------
            for p in range(NPAIR):
                for tb in range(NQB):
                    qp = ps512.tile([128, QBS], F32, tag="ps")
                    for k in range(NK):
                        nc.tensor.matmul(
                            qp[:],
                            lhsT=wq_sb[k][:, p * 128:(p + 1) * 128],
                            rhs=xT_sb[k][:, tbc(tb)],
                            start=(k == 0), stop=(k == NK - 1))
                    rope(qp, tb, [(slice(0, 128), qt_sb[p][:, tbc(tb)])])

            # ---------------- K projection + RoPE (into padded variants) ----
            for tb in range(NQB):
                kp = ps512.tile([128, QBS], F32, tag="ps")
                for k in range(NK):
                    nc.tensor.matmul(
                        kp[:], lhsT=wk_sb[k][:], rhs=xT_sb[k][:, tbc(tb)],
                        start=(k == 0), stop=(k == NK - 1))
                rope(kp, tb, [
                    (slice(0, 64), ktp[(0, 0)][0:64, tbc(tb)]),
                    (slice(64, 128), ktp[(1, 1)][64:128, tbc(tb)]),
                ])
            # zero pads + cross-partition copies
            nc.vector.memset(ktp[(0, 0)][64:128, :], 0.0)
            nc.vector.memset(ktp[(1, 1)][0:64, :], 0.0)
            nc.vector.memset(ktp[(1, 0)][0:64, :], 0.0)
            nc.vector.memset(ktp[(0, 1)][64:128, :], 0.0)
            nc.sync.dma_start(ktp[(1, 0)][64:128, :], ktp[(0, 0)][0:64, :])
            nc.sync.dma_start(ktp[(0, 1)][0:64, :], ktp[(1, 1)][64:128, :])

            # ---------------- V projection (natural layout) + ones column ---
            nc.vector.memset(va[0][:, :, D:D + 1], 1.0)
            nc.vector.memset(va[1][:, :, D:D + 1], 1.0)
            for t in range(NT):
                vp = ps512.tile([128, KF], F32, tag="ps")
                for k in range(NK):
                    nc.tensor.matmul(
                        vp[:], lhsT=xT_sb[k][:, t * 128:(t + 1) * 128],
                        rhs=wv_sb[k][:],
                        start=(k == 0), stop=(k == NK - 1))
                for v in (0, 1):
                    nc.vector.tensor_copy(
                        out=va[v][:, t, 0:D], in_=vp[:, v * D:(v + 1) * D])

            # ---------------- attention + o-proj ----------------
            for qb in range(NQB):
                for hh in range(QH):
                    p = hh // 2
                    row = 64 * (hh % 2)
                    v = hh // 4
                    ksel = ktp[(hh % 2, v)]
                    nkv = 4 * qb + 4
                    at = psAT.tile([D + 1, QBS], F32, tag="at")
                    for kv in range(nkv):
                        j = kv - 4 * qb
                        pt = wk.tile([128, QBS], BF16, tag="pt")
                        if j < 0:
                            sc = ps512.tile([128, QBS], F32, tag="ps")
                            nc.tensor.matmul(
                                sc[:], lhsT=ksel[:, kv * 128:(kv + 1) * 128],
                                rhs=qt_sb[p][:, tbc(qb)], start=True, stop=True)
                            nc.scalar.activation(pt[:], sc[:], AF.Exp)
                        else:
                            # q columns [0, 128j) of this kv tile are fully
                            # masked: zero them and only compute the rest.
                            c0 = 128 * j
                            w = QBS - c0
                            sc = ps512.tile([128, QBS], F32, tag="ps")
                            nc.tensor.matmul(
                                sc[:, 0:w], lhsT=ksel[:, kv * 128:(kv + 1) * 128],
                                rhs=qt_sb[p][:, qb * QBS + c0:(qb + 1) * QBS],
                                start=True, stop=True)
                            if c0:
                                nc.vector.memset(pt[:, 0:c0], 0.0)
                            pr = wk.tile([128, QBS], BF16, tag="pr")
                            nc.scalar.activation(pr[:, 0:w], sc[:, 0:w], AF.Exp)
                            nc.vector.tensor_tensor(
                                out=pt[:, c0:QBS], in0=pr[:, 0:w],
                                in1=msk[:, j * QBS + c0:(j + 1) * QBS], op=OP.mult)
                        nc.tensor.matmul(
                            at[:], lhsT=va[v][:, kv, :], rhs=pt[:],
                            start=(kv == 0), stop=(kv == nkv - 1))
                    # normalize by the ones-column sum
                    rec = wk2.tile([128, QBS], F32, tag="rec")
                    nc.vector.reciprocal(rec[D:D + 1, :], at[D:D + 1, :])
                    rec0 = wk2.tile([1, QBS], F32, tag="rec0")
                    nc.sync.dma_start(rec0[0:1, :], rec[D:D + 1, :])
                    rb = wk2.tile([64, QBS], F32, tag="rb")
                    nc.gpsimd.partition_broadcast(rb[:], rec0[0:1, :])
                    if row == 0:
                        nc.vector.tensor_tensor(
                            out=att[p][0:64, tbc(qb)], in0=at[0:D, :], in1=rb[:],
                            op=OP.mult)
                    else:
                        tmp = wk2.tile([64, QBS], BF16, tag="tmp64")
                        nc.vector.tensor_tensor(
                            out=tmp[:], in0=at[0:D, :], in1=rb[:], op=OP.mult)
                        nc.sync.dma_start(att[p][64:128, tbc(qb)], tmp[:])
                # o-proj for this q block's token tiles
                for tt in range(4):
                    t = qb * 4 + tt
                    for n in range(4):
                        op_ps = psO.tile([128, 512], F32, tag="op")
                        for p in range(NPAIR):
                            nc.tensor.matmul(
                                op_ps[:], lhsT=att[p][:, t * 128:(t + 1) * 128],
                                rhs=wo_sb[p][:, n * 512:(n + 1) * 512],
                                start=(p == 0), stop=(p == NPAIR - 1))
                        o_sb = wk.tile([128, 512], F32, tag="osb")
                        nc.any.tensor_copy(out=o_sb[:], in_=op_ps[:])
                        nc.sync.dma_start(
                            out[t * 128:(t + 1) * 128, n * 512:(n + 1) * 512],
                            o_sb[:])
            if dbg:
                nc.sync.dma_start(dbg["d_qt0"][:, :], qt_sb[0][:])
                nc.sync.dma_start(dbg["d_ktp00"][:, :], ktp[(0, 0)][:])
                nc.sync.dma_start(dbg["d_ktp10"][:, :], ktp[(1, 0)][:])
                nc.sync.dma_start(dbg["d_att0"][:, :], att[0][:])
                nc.sync.dma_start(dbg["d_att1"][:, :], att[1][:])
                nc.sync.dma_start(dbg["d_va0"][:, :], va[0][:, :, :])
    nc.compile()
    return nc


def _host_tables():
    freq = 1.0 / (10000.0 ** (np.arange(0, D, 2, dtype=np.float64) / D))
    t = np.arange(S, dtype=np.float64)
    fr = t[:, None] * freq[None, :]                       # (S, 32)
    emb = np.concatenate([fr, fr], axis=-1)               # (S, 64)
    cos64 = np.cos(emb).T.astype(np.float32)              # (64, S)
    sin64 = np.sin(emb).T.astype(np.float32)
    cos128 = np.concatenate([cos64, cos64], axis=0).astype(BF)
    sin128 = np.concatenate([sin64, sin64], axis=0).astype(BF)
    R = np.zeros((64, 64), np.float32)
    R[np.arange(32), 32 + np.arange(32)] = -1.0
    R[32 + np.arange(32), np.arange(32)] = 1.0
    R128 = np.zeros((128, 128), np.float32)
    R128[:64, :64] = R
    R128[64:, 64:] = R
    rotT = np.ascontiguousarray(R128.T).astype(BF)
    mask = np.zeros((128, 4 * QBS), np.float32)
    r = np.arange(128)[:, None]
    c = np.arange(QBS)[None, :]
    for j in range(4):
        mask[:, j * QBS:(j + 1) * QBS] = (128 * j + r <= c).astype(np.float32)
    return cos128, sin128, rotT, mask.astype(BF)


def kernel(x, Wq, Wk, Wv, Wo):
    x = np.asarray(x, np.float32)
    Wq = np.asarray(Wq, np.float32)
    Wk = np.asarray(Wk, np.float32)
    Wv = np.asarray(Wv, np.float32)
    Wo = np.asarray(Wo, np.float32)

    if "nc" not in _CACHE:
        _CACHE["nc"] = _build_program()
    nc = _CACHE["nc"]

    cos128, sin128, rotT, maskb = _host_tables()
    in_maps = []
    for core in range(8):
        g, b = core // 2, core % 2
        im = {
            "xT": np.ascontiguousarray(x[b].T).astype(BF),
            "wqT": np.ascontiguousarray((Wq[QF * g:QF * (g + 1), :] / 8.0).T).astype(BF),
            "wkT": np.ascontiguousarray(Wk[KF * g:KF * (g + 1), :].T).astype(BF),
            "wvT": np.ascontiguousarray(Wv[KF * g:KF * (g + 1), :].T).astype(BF),
            "woT": np.ascontiguousarray(Wo[:, QF * g:QF * (g + 1)].T).astype(BF),
            "cost": cos128,
            "sint": sin128,
            "rotT": rotT,
            "maskd": maskb,
        }
        in_maps.append(im)

    trace = bool(int(os.environ.get("KERNEL_TRACE", "0")))
    res = bass_utils.run_bass_kernel_spmd(
        nc, in_maps, core_ids=list(range(8)), trace=trace)
    _CACHE["last_result"] = res

    out = np.zeros((B, S, H), np.float32)
    for core in range(8):
        g, b = core // 2, core % 2
        out[b] += np.asarray(res.results[core]["out"], np.float32)
    return out

